# revision 52
# baseline (speedup 1.0000x reference)
"""Phi4 differential flash-attention block on 8 trn2 NeuronCores.

Sharding: 2-way sequence (stride-2 interleave) x 4-way head-pair tensor
parallel. Core c handles seq group g = c % 2 (query rows g::2) and head
group hg = c // 2 (5 differential head pairs, one KV pair). Each core
computes K/V for the full sequence (its KV pair only), Q for its own
rows, flash attention in transposed-score layout (scoresT = [keys, q]),
the differential combine + rmsnorm, and a partial output projection.
The host sums the 4 head-group partials per seq group and adds out_b.

Implementation notes:
  - fp16 on the whole matmul path (pipelined LDWEIGHTS + FWL; fp32 PSUM
    accumulation), biases/softmax stats in fp32.
  - hidden-state chunks live resident in SBUF so every projection pass
    streams matmuls back-to-back (PE stays warm, one PSUM bank).
  - QK processes two head pairs per matmul (shared kT stationary, N=512).
  - softmax denominator via an appended ones-column in V (no reduction).
  - subln weight and (1 - lambda_init) folded into out_w on the host.
  - rmsnorm rsqrt deferred and batched: one Ln + one Exp over all 40
    row-groups, so ACT never thrashes activation-table sets.
"""
import math
import os

import numpy as np

import concourse.bacc as bacc
import concourse.tile as tile
import concourse.mybir as mybir
from concourse.bass import ds, ts
from concourse.masks import make_identity
from concourse.bass_utils import run_bass_kernel_spmd

f32 = mybir.dt.float32
f16 = mybir.dt.float16
AF = mybir.ActivationFunctionType
OP = mybir.AluOpType

# Problem constants (hardcoded per harness contract)
S, H, NH, NKV, D = 2048, 2560, 40, 4, 64
LAYER_IDX = 17
LAMBDA_INIT = 0.8 - 0.6 * math.exp(-0.3 * LAYER_IDX)
SCALE = 1.0 / math.sqrt(D)
P = 128
HT = H // P            # 20 h-tiles
N_CORES = 8
N_SEQ = 2              # sequence groups (stride-2)
N_HG = 4               # head groups
PAIRS = 5              # head pairs per core
S_LOC = S // N_SEQ     # 1024 own queries per core
QT = 256               # queries per attention supertile
N_JT = S_LOC // QT     # 4
QB = S_LOC // P        # 8 own query blocks
EPS = 1e-5

_PROGRAM = None


def _build_program(sim_compat=False):
    nc = bacc.Bacc()

    hidT_kv = nc.dram_tensor("hidT_kv", [H, S], f16, kind="ExternalInput")
    wkvT = nc.dram_tensor("wkvT", [H, 2 * P], f16, kind="ExternalInput")
    wqT = nc.dram_tensor("wqT", [H, PAIRS * P], f16, kind="ExternalInput")
    owT = nc.dram_tensor("owT", [PAIRS * P, H], f16, kind="ExternalInput")
    bkv = nc.dram_tensor("bkv", [P, 2], f32, kind="ExternalInput")
    bq = nc.dram_tensor("bq", [P, PAIRS], f32, kind="ExternalInput")
    maskd = nc.dram_tensor("maskd", [P, 4, QT], f16, kind="ExternalInput")
    lam = nc.dram_tensor("lam", [1, 1], f32, kind="ExternalInput")
    out = nc.dram_tensor("out", [S_LOC, H], f16, kind="ExternalOutput")

    hkv_v = hidT_kv[:].rearrange("(ho p) s -> p ho s", p=P)       # [128,20,2048]
    wkv_v = wkvT[:].rearrange("(ho p) f -> p ho f", p=P)          # [128,20,256]
    wq_v = wqT[:].rearrange("(ho p) f -> p ho f", p=P)            # [128,20,640]
    ow_v = owT[:].rearrange("(pt p) h -> pt p h", p=P)            # [5,128,2560]
    out_v = out[:].rearrange("(qb p) h -> qb p h", p=P)           # [8,128,2560]

    with tile.TileContext(nc) as tc:
        with (
            tc.tile_pool(name="singles", bufs=1) as singles,
            tc.tile_pool(name="hres", bufs=1) as hres,
            tc.tile_pool(name="etile", bufs=2) as etile,
            tc.tile_pool(name="tpool", bufs=2) as tpool,
            tc.tile_pool(name="opool", bufs=4) as opool,
            tc.tile_pool(name="pj", bufs=1, space="PSUM") as pj,
            tc.tile_pool(name="ptr", bufs=1, space="PSUM") as ptr,
        ):
            # ---------- critical-path DMAs first ----------
            # Everything phase 1+2 needs rides the SP queue just-in-time:
            # wkv, big-chunk 0 (finely sliced so the first KV pass starts
            # as soon as the first h-slab lands), wq, then big-chunk 1.
            # mask/ow go out on the Activation HWDGE queue, dispatched only
            # after the first KV chunk is consumed so they never steal
            # bandwidth from the critical stream.
            wkv_sb = singles.tile([P, HT, 2 * P], f16)
            hch_bc = [hres.tile([P, HT, 1024], f16, name=f"hch{b}")
                      for b in range(2)]
            nc.sync.dma_start(wkv_sb[:, ds(0, 5), :], wkv_v[:, ds(0, 5), :])
            nc.sync.dma_start(hch_bc[0][:, ds(0, 5), ds(0, 512)],
                              hkv_v[:, ds(0, 5), ds(0, 512)])
            for hg in range(1, 4):
                nc.sync.dma_start(wkv_sb[:, ds(hg * 5, 5), :],
                                  wkv_v[:, ds(hg * 5, 5), :])
                nc.sync.dma_start(hch_bc[0][:, ds(hg * 5, 5), ds(0, 512)],
                                  hkv_v[:, ds(hg * 5, 5), ds(0, 512)])
            bkv_sb = singles.tile([P, 2], f32)
            nc.sync.dma_start(bkv_sb[:], bkv[:])
            bq_sb = singles.tile([P, PAIRS], f32)
            nc.sync.dma_start(bq_sb[:], bq[:])
            wq_sb = singles.tile([P, HT, PAIRS * P], f16)

            mask_sb = singles.tile([P, 4, QT], f16)
            ow_sb = [singles.tile([P, H], f16, name=f"ow{pt_}")
                     for pt_ in range(PAIRS)]

            # ---------- resident constants ----------
            ident = singles.tile([P, P], f16)
            make_identity(nc, ident)
            lam_sb = singles.tile([P, 1], f32)
            nc.sync.dma_start(lam_sb[:], lam[:].partition_broadcast(P))
            eps_sb = singles.tile([P, 1], f32)
            nc.vector.memset(eps_sb[:], EPS)

            # ---------- resident activations ----------
            kT = singles.tile([P, S], f16)               # [k1|k2, seq]
            vT = singles.tile([P, S], f16)               # [v1|v2, seq]
            v_sb = singles.tile([P, S // P, 132], f16)   # [keys, kb, v1|v2|1s]
            nc.vector.memset(v_sb[:], 0.0)
            nc.vector.memset(v_sb[:, :, 128:129], 1.0)
            qTall = singles.tile([P, PAIRS, S_LOC], f16)  # [q1|q2, pair, seq]
            xat = [singles.tile([P, QB, P], f16, name=f"xat{p}")
                   for p in range(PAIRS)]                # unnormalized attn rows
            ms_all = singles.tile([P, QB, PAIRS], f32)   # row sum-of-squares
            xhatT = [singles.tile([P, QB, P], f16, name=f"xhatT{p}")
                     for p in range(PAIRS)]

            def v_transpose(kb):
                pvt = ptr.tile([P, P], f16, tag="pt", name="pvt")
                nc.tensor.transpose(pvt[:], vT[:, ts(kb, P)], ident[:])
                nc.vector.tensor_copy(v_sb[:, kb, 0:128], pvt[:])

            def q_fill(sh, p, pool):
                # own-query columns are the even columns of the big chunk
                pp = pool.tile([P, 512], f32, tag="pj", name="pp")
                for h in range(HT):
                    nc.tensor.matmul(pp[:], wq_sb[:, h, ts(p, P)],
                                     hch_bc[sh][:, h, ds(0, 512, 2)],
                                     start=(h == 0), stop=(h == HT - 1))
                nc.scalar.activation(qTall[:, p, ds(sh * 512, 512)], pp[:],
                                     AF.Identity, bias=bq_sb[:, p:p + 1])

            # ---------- phase 1+2: K/V projection (full seq), Q (sh=0) ----
            # Later transfers (B1, wq, mask, ow) are dispatched from the
            # Activation engine at points it reaches only mid-phase: the DMA
            # queue runs its transfers concurrently with shared bandwidth,
            # so anything in flight early would slow the critical stream.
            with tc.tile_pool(name="pjA", bufs=2, space="PSUM") as pjA:
                def kv_pass(bc, half, f):
                    dest = kT if f == 0 else vT
                    pp = pjA.tile([P, 512], f32, tag="pj", name="pp")
                    for h in range(HT):
                        nc.tensor.matmul(
                            pp[:], wkv_sb[:, h, ds(f * P, P)],
                            hch_bc[bc][:, h, ds(half * 512, 512)],
                            start=(h == 0), stop=(h == HT - 1))
                    nc.scalar.activation(
                        dest[:, ds(bc * 1024 + half * 512, 512)],
                        pp[:], AF.Identity, bias=bkv_sb[:, f:f + 1])

                for half in range(2):
                    for f in range(2):
                        kv_pass(0, half, f)
                        if half == 0 and f == 0:
                            # B0-half1 slabs paced from here (Act reaches
                            # this only after the first k pass) so the
                            # fair-shared DMA engines stay focused on the
                            # half-0 slabs being consumed right now
                            for hgs in range(4):
                                nc.scalar.dma_start(
                                    hch_bc[0][:, ds(hgs * 5, 5),
                                              ds(512, 512)],
                                    hkv_v[:, ds(hgs * 5, 5),
                                          ds(512, 512)])
                        if half == 1:
                            # B1 + wq slabs, paced per activation point
                            for s2 in range(4 * f, 4 * f + 4):
                                hgs, hf = s2 % 4, s2 // 4
                                nc.scalar.dma_start(
                                    hch_bc[1][:, ds(hgs * 5, 5),
                                              ds(hf * 512, 512)],
                                    hkv_v[:, ds(hgs * 5, 5),
                                          ds(1024 + hf * 512, 512)])
                            for p5 in range(3 * f, 3 * f + 3):
                                if p5 < PAIRS:
                                    nc.scalar.dma_start(
                                        wq_sb[:, :, ts(p5, P)],
                                        wq_v[:, :, ts(p5, P)])
                nc.scalar.dma_start(mask_sb[:], maskd[:])
                for pt_ in range(PAIRS):
                    nc.scalar.dma_start(ow_sb[pt_][:], ow_v[pt_])
                # interleave Q (sh=0) with the KV(B1) passes: Q reads only
                # resident bytes, so it fills the DMA-paced stretches of B1
                vts = {0: (0, 1), 1: (2, 3), 2: (4, 5, 8, 9),
                       3: (6, 7, 10, 11), 4: (12, 13, 14, 15)}
                for p in range(PAIRS):
                    q_fill(0, p, pjA)
                    if p < 4:
                        kv_pass(1, p // 2, p % 2)
                    for kb in vts[p]:
                        v_transpose(kb)

            def _rsqrt_chain(csl, n):
                # DVE-only inverse sqrt (bit-trick seed + 2 Newton steps) so
                # the scalar engine never leaves the exp table set.
                i32 = mybir.dt.int32
                v = tpool.tile([P, n], f32, name="vms")
                nc.vector.tensor_scalar(v[:], ms_all[csl], 1.0 / P, EPS,
                                        OP.mult, OP.add)
                hv = tpool.tile([P, n], f32, name="hv")
                nc.vector.tensor_scalar_mul(hv[:], v[:], 0.5)
                fb = tpool.tile([P, n], f32, name="fb")
                nc.vector.tensor_copy(fb[:], v[:].bitcast(i32))  # int bits -> f32
                nc.vector.tensor_scalar(fb[:], fb[:], -0.5, 1597463007.0,
                                        OP.mult, OP.add)
                yi = tpool.tile([P, n], i32, name="yi")
                nc.vector.tensor_copy(yi[:], fb[:])              # f32 -> int bits
                y = tpool.tile([P, n], f32, name="yrs")
                nc.vector.tensor_copy(y[:], yi[:].bitcast(f32))
                t = tpool.tile([P, n], f32, name="trs")
                for _ in range(2):                               # Newton
                    nc.vector.tensor_tensor(t[:], y[:], y[:], OP.mult)
                    nc.vector.tensor_tensor(t[:], t[:], hv[:], OP.mult)
                    nc.vector.tensor_scalar(t[:], t[:], -1.0, 1.5,
                                            OP.mult, OP.add)
                    nc.vector.tensor_tensor(y[:], y[:], t[:], OP.mult)
                nc.vector.tensor_copy(ms_all[csl], y[:])

            def _xhat_scale(jt, p, qs):
                qb = jt * 2 + qs
                xh = tpool.tile([P, P], f16, name="xh")
                nc.vector.tensor_scalar_mul(xh[:], xat[p][:, qb, :],
                                            ms_all[:, qb, p:p + 1])
                pt = ptr.tile([P, P], f16, tag="pt", name="pt")
                nc.tensor.transpose(pt[:], xh[:], ident[:])
                nc.vector.tensor_copy(xhatT[p][:, qb, :], pt[:])

            def normalize_jt(jt):
                # batched rmsnorm rsqrt for this supertile's 10 row-groups
                _rsqrt_chain((slice(None), ds(2 * jt, 2), slice(None)),
                             2 * PAIRS)
                for qs in range(2):
                    for p in range(PAIRS):
                        _xhat_scale(jt, p, qs)

            def normalize_pair(jt, p):
                # per-pair variant (last supertile): xhatT finishes pair by
                # pair so the tail out-proj fills start without a stall
                _rsqrt_chain((slice(None), ds(2 * jt, 2), slice(p, p + 1)), 2)
                for qs in range(2):
                    _xhat_scale(jt, p, qs)

            def oproj_fill(qb, hc, pool, on_vec=False):
                po = pool.tile([P, 512], f32, tag="pj", name="po")
                for p in range(PAIRS):
                    nc.tensor.matmul(po[:], xhatT[p][:, qb, :],
                                     ow_sb[p][:, ds(hc * 512, 512)],
                                     start=(p == 0), stop=(p == PAIRS - 1))
                ot = opool.tile([P, 512], f16, name="ot")
                if on_vec:
                    nc.vector.tensor_copy(ot[:], po[:])
                else:
                    nc.scalar.copy(ot[:], po[:])
                nc.sync.dma_start(out_v[qb][:, ds(hc * 512, 512)], ot[:])

            # ---------- phase 3: attention ----------
            with (
                tc.tile_pool(name="psc", bufs=1, space="PSUM") as psc,
                tc.tile_pool(name="pav", bufs=1, space="PSUM") as pav,
            ):
                for jt in range(N_JT):
                    E = 4 * (jt + 1)          # key blocks for this supertile
                    prev_fills = ([(2 * (jt - 1) + f % 2, f // 2)
                                   for f in range(10)] if jt > 0 else [])
                    for p in range(PAIRS):
                        if jt == 0:
                            q_fill(1, p, pj)
                        if prev_fills:
                            oproj_fill(*prev_fills.pop(0), pj)
                            oproj_fill(*prev_fills.pop(0), pj, on_vec=True)
                        if sim_compat:
                            avt = [[pav.tile([P, 132], f32, tag=f"av{s}{q}",
                                             name=f"av{s}{q}") for q in range(2)]
                                   for s in range(2)]

                            def avap(s, qs, lo, n, avt=avt):
                                return avt[s][qs][:, ds(lo, n)]
                        else:
                            avt = [pav.tile([P, 264], f32, tag=f"av{s}",
                                            name=f"av{s}") for s in range(2)]

                            def avap(s, qs, lo, n, avt=avt):
                                return avt[s][:, ds(132 * qs + lo, n)]

                        def emit_av(ev, kbs):
                            for j, kb in enumerate(kbs):
                                for qs in range(2):
                                    st = kb == 0 and (qs == 0 or sim_compat)
                                    sp = kb == E - 1
                                    for s in range(2):
                                        nc.tensor.matmul(
                                            avap(s, qs, 0, 132),
                                            ev[:, s, j, ds(128 * qs, 128)],
                                            v_sb[:, kb, :], start=st, stop=sp)

                        pend = None
                        for grp in range(E // 4):     # 4 key blocks per group
                            kbs = tuple(range(4 * grp, 4 * grp + 4))
                            ee = etile.tile([P, 2, 4, QT], f16, name="ee")
                            rhs1 = qTall[0:64, p, ds(jt * QT, QT)]
                            rhs2 = qTall[64:128, p, ds(jt * QT, QT)]
                            for s, (ksl, rhs) in enumerate(
                                    ((slice(0, 64), rhs1),
                                     (slice(64, 128), rhs2))):
                                sc = psc.tile([P, 4, QT], f32, tag=f"s{s}",
                                              name=f"s{s}")
                                for j, kb in enumerate(kbs):
                                    nc.tensor.matmul(sc[:, j, :],
                                                     kT[ksl, ts(kb, P)],
                                                     rhs, start=True, stop=True)
                                nc.scalar.activation(ee[:, s], sc[:],
                                                     AF.Exp, scale=SCALE)
                            if grp == jt:             # last group = diag band
                                for s in range(2):
                                    nc.vector.tensor_tensor(ee[:, s], ee[:, s],
                                                            mask_sb[:], OP.mult)
                            if pend is not None:
                                emit_av(*pend)
                            pend = (ee, kbs)
                        emit_av(*pend)
                        # epilogue: combine + row sum-of-squares (rsqrt deferred)
                        rec = tpool.tile([P, 4], f32, name="rec")
                        if sim_compat:
                            for qs in range(2):
                                nc.vector.reciprocal(rec[:, qs:qs + 1],
                                                     avap(0, qs, 128, 1))
                                nc.vector.reciprocal(rec[:, 2 + qs:3 + qs],
                                                     avap(1, qs, 128, 1))
                        else:
                            for s in range(2):
                                nc.vector.reciprocal(
                                    rec[:, 2 * s:2 * s + 2],
                                    avt[s][:, ds(128, 2, 132)])
                        nc.vector.tensor_scalar_mul(rec[:, 2:4], rec[:, 2:4],
                                                    lam_sb[:])
                        for qs in range(2):
                            qb = jt * 2 + qs
                            xs = xat[p][:, qb, :]
                            xb = tpool.tile([P, P], f16, name="xb")
                            nc.vector.tensor_scalar_mul(
                                xs, avap(0, qs, 0, P), rec[:, qs:qs + 1])
                            nc.vector.tensor_scalar_mul(
                                xb[:], avap(1, qs, 0, P),
                                rec[:, 2 + qs:3 + qs])
                            nc.vector.tensor_tensor(xs, xs, xb[:], OP.subtract)
                            sq = tpool.tile([P, P], f32, name="sq")
                            nc.vector.tensor_tensor(sq[:], xs, xs, OP.mult)
                            nc.vector.reduce_sum(ms_all[:, qb, p:p + 1], sq[:],
                                                 axis=mybir.AxisListType.X)
                        if jt == N_JT - 1:
                            normalize_pair(jt, p)
                    if jt < N_JT - 1:
                        normalize_jt(jt)

            # ---------- final out-proj fills (last supertile) ----------
            with tc.tile_pool(name="pjT", bufs=3, space="PSUM") as pjT:
                for qb in (2 * (N_JT - 1), 2 * (N_JT - 1) + 1):
                    for hc in range(H // 512):
                        oproj_fill(qb, hc, pjT, on_vec=(hc % 2 == 0))

    nc.compile()
    return nc


def _prep_inputs(hidden_states, Wqkv_w, Wqkv_b, out_w, out_b,
                 lambda_q1, lambda_k1, lambda_q2, lambda_k2, subln_w):
    hs = np.asarray(hidden_states, np.float32).reshape(S, H)
    Wqkv_w = np.asarray(Wqkv_w, np.float32)
    Wqkv_b = np.asarray(Wqkv_b, np.float32)
    out_w = np.asarray(out_w, np.float32)
    subln_w = np.asarray(subln_w, np.float32)

    lam_full = np.float32(
        np.exp(np.dot(np.asarray(lambda_q1, np.float64),
                      np.asarray(lambda_k1, np.float64)))
        - np.exp(np.dot(np.asarray(lambda_q2, np.float64),
                        np.asarray(lambda_k2, np.float64)))
        + LAMBDA_INIT)

    # Per seq-group hidden layout: pair-swap the sequence for g=1 so own
    # query columns always sit at even positions (one SPMD program). The
    # keys then appear pair-swapped too, which only the causal mask (also
    # per-core host data) needs to know about.
    hidTs, masks = [], []
    kk = np.arange(P)[:, None, None]
    bb = np.arange(4)[None, :, None]
    ii = np.arange(QT)[None, None, :]
    for g in range(N_SEQ):
        hidTs.append(np.ascontiguousarray(
            hs[np.arange(S) ^ g].T.astype(np.float16)))              # [H, S]
        masks.append(
            ((2 * ii + g) >= ((128 * bb + kk) ^ g)).astype(np.float16))

    in_maps = []
    for c in range(N_CORES):
        g, hg = c % N_SEQ, c // N_SEQ
        kp = hg // 2                                 # kv pair for this head group
        krows = slice(H + P * kp, H + P * (kp + 1))
        vrows = slice(H + NKV * D + P * kp, H + NKV * D + P * (kp + 1))
        qrows = slice(640 * hg, 640 * (hg + 1))
        wq = np.ascontiguousarray(Wqkv_w[qrows].T.astype(np.float16))   # [H, 640]
        wkv = np.ascontiguousarray(np.concatenate(
            [Wqkv_w[krows].T, Wqkv_w[vrows].T], axis=1).astype(np.float16))
        bkv = np.ascontiguousarray(
            np.stack([Wqkv_b[krows], Wqkv_b[vrows]], axis=1))            # [128,2]
        bq = np.ascontiguousarray(Wqkv_b[qrows].reshape(PAIRS, P).T)     # [128,5]
        sub = np.tile(subln_w, PAIRS) * (1.0 - LAMBDA_INIT)              # [640]
        ow = np.ascontiguousarray(
            (out_w[:, qrows].T * sub[:, None]).astype(np.float16))       # [640,H]
        in_maps.append({
            "hidT_kv": hidTs[g],
            "wkvT": wkv,
            "wqT": wq,
            "owT": ow,
            "bkv": bkv,
            "bq": bq,
            "maskd": masks[g],
            "lam": np.array([[lam_full]], np.float32),
        })
    return in_maps


def run(inputs, trace=False):
    global _PROGRAM
    if _PROGRAM is None:
        _PROGRAM = _build_program(
            sim_compat=os.environ.get("KSIMCOMPAT", "0") == "1")
    in_maps = _prep_inputs(**inputs)
    res = run_bass_kernel_spmd(_PROGRAM, in_maps,
                               core_ids=list(range(N_CORES)), trace=trace)
    out_b = np.asarray(inputs["out_b"], np.float32)
    full = np.empty((S, H), np.float32)
    for g in range(N_SEQ):
        acc = np.zeros((S_LOC, H), np.float32)
        for hg in range(N_HG):
            acc += np.asarray(res.results[hg * N_SEQ + g]["out"], np.float32)
        full[g::2] = acc + out_b
    return full.reshape(1, S, H), res


def kernel(**inputs):
    return run(inputs, trace=False)[0]



# revision 53
# speedup vs baseline: 1.0197x; 1.0197x over previous
"""Phi4 differential flash-attention block on 8 trn2 NeuronCores.

Sharding: 2-way sequence (stride-2 interleave) x 4-way head-pair tensor
parallel. Core c handles seq group g = c % 2 (query rows g::2) and head
group hg = c // 2 (5 differential head pairs, one KV pair). Each core
computes K/V for the full sequence (its KV pair only), Q for its own
rows, flash attention in transposed-score layout (scoresT = [keys, q]),
the differential combine + rmsnorm, and a partial output projection.
The host sums the 4 head-group partials per seq group and adds out_b.

Implementation notes:
  - fp16 on the whole matmul path (pipelined LDWEIGHTS + FWL; fp32 PSUM
    accumulation), biases/softmax stats in fp32.
  - hidden-state chunks live resident in SBUF so every projection pass
    streams matmuls back-to-back (PE stays warm, one PSUM bank).
  - QK processes two head pairs per matmul (shared kT stationary, N=512).
  - softmax denominator via an appended ones-column in V (no reduction).
  - subln weight and (1 - lambda_init) folded into out_w on the host.
  - rmsnorm rsqrt deferred and batched: one Ln + one Exp over all 40
    row-groups, so ACT never thrashes activation-table sets.
"""
import math
import os

import numpy as np

import concourse.bacc as bacc
import concourse.tile as tile
import concourse.mybir as mybir
from concourse.bass import ds, ts
from concourse.masks import make_identity
from concourse.bass_utils import run_bass_kernel_spmd

f32 = mybir.dt.float32
f16 = mybir.dt.float16
AF = mybir.ActivationFunctionType
OP = mybir.AluOpType

# Problem constants (hardcoded per harness contract)
S, H, NH, NKV, D = 2048, 2560, 40, 4, 64
LAYER_IDX = 17
LAMBDA_INIT = 0.8 - 0.6 * math.exp(-0.3 * LAYER_IDX)
SCALE = 1.0 / math.sqrt(D)
P = 128
HT = H // P            # 20 h-tiles
N_CORES = 8
N_SEQ = 2              # sequence groups (stride-2)
N_HG = 4               # head groups
PAIRS = 5              # head pairs per core
S_LOC = S // N_SEQ     # 1024 own queries per core
QT = 256               # queries per attention supertile
N_JT = S_LOC // QT     # 4
QB = S_LOC // P        # 8 own query blocks
EPS = 1e-5

_PROGRAM = None


def _build_program(sim_compat=False):
    nc = bacc.Bacc()

    hidT_kv = nc.dram_tensor("hidT_kv", [H, S], f16, kind="ExternalInput")
    wkvT = nc.dram_tensor("wkvT", [H, 2 * P], f16, kind="ExternalInput")
    wqT = nc.dram_tensor("wqT", [H, PAIRS * P], f16, kind="ExternalInput")
    owT = nc.dram_tensor("owT", [PAIRS * P, H], f16, kind="ExternalInput")
    bkv = nc.dram_tensor("bkv", [P, 2], f32, kind="ExternalInput")
    bq = nc.dram_tensor("bq", [P, PAIRS], f32, kind="ExternalInput")
    maskd = nc.dram_tensor("maskd", [P, 4, QT], f16, kind="ExternalInput")
    lam = nc.dram_tensor("lam", [1, 1], f32, kind="ExternalInput")
    out = nc.dram_tensor("out", [S_LOC, H], f16, kind="ExternalOutput")

    hkv_v = hidT_kv[:].rearrange("(ho p) s -> p ho s", p=P)       # [128,20,2048]
    wkv_v = wkvT[:].rearrange("(ho p) f -> p ho f", p=P)          # [128,20,256]
    wq_v = wqT[:].rearrange("(ho p) f -> p ho f", p=P)            # [128,20,640]
    ow_v = owT[:].rearrange("(pt p) h -> pt p h", p=P)            # [5,128,2560]
    out_v = out[:].rearrange("(qb p) h -> qb p h", p=P)           # [8,128,2560]

    with tile.TileContext(nc) as tc:
        with (
            tc.tile_pool(name="singles", bufs=1) as singles,
            tc.tile_pool(name="hres", bufs=1) as hres,
            tc.tile_pool(name="etile", bufs=2) as etile,
            tc.tile_pool(name="tpool", bufs=2) as tpool,
            tc.tile_pool(name="opool", bufs=4) as opool,
            tc.tile_pool(name="pj", bufs=1, space="PSUM") as pj,
            tc.tile_pool(name="ptr", bufs=1, space="PSUM") as ptr,
        ):
            # ---------- critical-path DMAs first ----------
            # Everything phase 1+2 needs rides the SP queue just-in-time:
            # wkv, big-chunk 0 (finely sliced so the first KV pass starts
            # as soon as the first h-slab lands), wq, then big-chunk 1.
            # mask/ow go out on the Activation HWDGE queue, dispatched only
            # after the first KV chunk is consumed so they never steal
            # bandwidth from the critical stream.
            wkv_sb = singles.tile([P, HT, 2 * P], f16)
            hch_bc = [hres.tile([P, HT, 1024], f16, name=f"hch{b}")
                      for b in range(2)]
            nc.sync.dma_start(wkv_sb[:, ds(0, 5), :], wkv_v[:, ds(0, 5), :])
            nc.sync.dma_start(hch_bc[0][:, ds(0, 5), ds(0, 512)],
                              hkv_v[:, ds(0, 5), ds(0, 512)])
            for hg in range(1, 4):
                nc.sync.dma_start(wkv_sb[:, ds(hg * 5, 5), :],
                                  wkv_v[:, ds(hg * 5, 5), :])
                nc.sync.dma_start(hch_bc[0][:, ds(hg * 5, 5), ds(0, 512)],
                                  hkv_v[:, ds(hg * 5, 5), ds(0, 512)])
            bkv_sb = singles.tile([P, 2], f32)
            nc.sync.dma_start(bkv_sb[:], bkv[:])
            bq_sb = singles.tile([P, PAIRS], f32)
            nc.sync.dma_start(bq_sb[:], bq[:])
            wq_sb = singles.tile([P, HT, PAIRS * P], f16)

            mask_sb = singles.tile([P, 4, QT], f16)
            ow_sb = [singles.tile([P, H], f16, name=f"ow{pt_}")
                     for pt_ in range(PAIRS)]

            # ---------- resident constants ----------
            ident = singles.tile([P, P], f16)
            make_identity(nc, ident)
            lam_sb = singles.tile([P, 1], f32)
            nc.sync.dma_start(lam_sb[:], lam[:].partition_broadcast(P))
            eps_sb = singles.tile([P, 1], f32)
            nc.vector.memset(eps_sb[:], EPS)

            # ---------- resident activations ----------
            kT = singles.tile([P, S], f16)               # [k1|k2, seq]
            vT = singles.tile([P, S], f16)               # [v1|v2, seq]
            v_sb = singles.tile([P, S // P, 132], f16)   # [keys, kb, v1|v2|1s]
            nc.vector.memset(v_sb[:], 0.0)
            nc.vector.memset(v_sb[:, :, 128:129], 1.0)
            qTall = singles.tile([P, PAIRS, S_LOC], f16)  # [q1|q2, pair, seq]
            xat = [singles.tile([P, QB, P], f16, name=f"xat{p}")
                   for p in range(PAIRS)]                # unnormalized attn rows
            ms_all = singles.tile([P, QB, PAIRS], f32)   # row sum-of-squares
            xhatT = [singles.tile([P, QB, P], f16, name=f"xhatT{p}")
                     for p in range(PAIRS)]

            def v_transpose(kb):
                pvt = ptr.tile([P, P], f16, tag="pt", name="pvt")
                nc.tensor.transpose(pvt[:], vT[:, ts(kb, P)], ident[:])
                nc.vector.tensor_copy(v_sb[:, kb, 0:128], pvt[:])

            def q_fill(sh, p, pool):
                # own-query columns are the even columns of the big chunk
                pp = pool.tile([P, 512], f32, tag="pj", name="pp")
                for h in range(HT):
                    nc.tensor.matmul(pp[:], wq_sb[:, h, ts(p, P)],
                                     hch_bc[sh][:, h, ds(0, 512, 2)],
                                     start=(h == 0), stop=(h == HT - 1))
                nc.scalar.activation(qTall[:, p, ds(sh * 512, 512)], pp[:],
                                     AF.Identity, bias=bq_sb[:, p:p + 1])

            # ---------- phase 1+2: K/V projection (full seq), Q (sh=0) ----
            # Later transfers (B1, wq, mask, ow) are dispatched from the
            # Activation engine at points it reaches only mid-phase: the DMA
            # queue runs its transfers concurrently with shared bandwidth,
            # so anything in flight early would slow the critical stream.
            with tc.tile_pool(name="pjA", bufs=2, space="PSUM") as pjA:
                def kv_pass(bc, half, f):
                    dest = kT if f == 0 else vT
                    pp = pjA.tile([P, 512], f32, tag="pj", name="pp")
                    for h in range(HT):
                        nc.tensor.matmul(
                            pp[:], wkv_sb[:, h, ds(f * P, P)],
                            hch_bc[bc][:, h, ds(half * 512, 512)],
                            start=(h == 0), stop=(h == HT - 1))
                    nc.scalar.activation(
                        dest[:, ds(bc * 1024 + half * 512, 512)],
                        pp[:], AF.Identity, bias=bkv_sb[:, f:f + 1])

                for half in range(2):
                    for f in range(2):
                        kv_pass(0, half, f)
                        if half == 0 and f == 0:
                            # B0-half1 slabs paced from here (Act reaches
                            # this only after the first k pass) so the
                            # fair-shared DMA engines stay focused on the
                            # half-0 slabs being consumed right now
                            for hgs in range(4):
                                nc.scalar.dma_start(
                                    hch_bc[0][:, ds(hgs * 5, 5),
                                              ds(512, 512)],
                                    hkv_v[:, ds(hgs * 5, 5),
                                          ds(512, 512)])
                        if half == 1:
                            # B1 + wq slabs, paced per activation point
                            for s2 in range(4 * f, 4 * f + 4):
                                hgs, hf = s2 % 4, s2 // 4
                                nc.scalar.dma_start(
                                    hch_bc[1][:, ds(hgs * 5, 5),
                                              ds(hf * 512, 512)],
                                    hkv_v[:, ds(hgs * 5, 5),
                                          ds(1024 + hf * 512, 512)])
                            for p5 in range(3 * f, 3 * f + 3):
                                if p5 < PAIRS:
                                    nc.scalar.dma_start(
                                        wq_sb[:, :, ts(p5, P)],
                                        wq_v[:, :, ts(p5, P)])
                nc.scalar.dma_start(mask_sb[:], maskd[:])
                for pt_ in range(PAIRS):
                    nc.scalar.dma_start(ow_sb[pt_][:], ow_v[pt_])
                # interleave Q (sh=0) with the KV(B1) passes: Q reads only
                # resident bytes, so it fills the DMA-paced stretches of B1
                vts = {0: (0, 1), 1: (2, 3), 2: (4, 5, 8, 9),
                       3: (6, 7, 10, 11), 4: (12, 13, 14, 15)}
                for p in range(PAIRS):
                    q_fill(0, p, pjA)
                    if p < 4:
                        kv_pass(1, p // 2, p % 2)
                    for kb in vts[p]:
                        v_transpose(kb)

            def _rsqrt_chain(csl, n):
                # DVE-only inverse sqrt (bit-trick seed + 2 Newton steps) so
                # the scalar engine never leaves the exp table set.
                i32 = mybir.dt.int32
                v = tpool.tile([P, n], f32, name="vms")
                nc.vector.tensor_scalar(v[:], ms_all[csl], 1.0 / P, EPS,
                                        OP.mult, OP.add)
                hv = tpool.tile([P, n], f32, name="hv")
                nc.vector.tensor_scalar_mul(hv[:], v[:], 0.5)
                fb = tpool.tile([P, n], f32, name="fb")
                nc.vector.tensor_copy(fb[:], v[:].bitcast(i32))  # int bits -> f32
                nc.vector.tensor_scalar(fb[:], fb[:], -0.5, 1597463007.0,
                                        OP.mult, OP.add)
                yi = tpool.tile([P, n], i32, name="yi")
                nc.vector.tensor_copy(yi[:], fb[:])              # f32 -> int bits
                y = tpool.tile([P, n], f32, name="yrs")
                nc.vector.tensor_copy(y[:], yi[:].bitcast(f32))
                t = tpool.tile([P, n], f32, name="trs")
                for _ in range(2):                               # Newton
                    nc.vector.tensor_tensor(t[:], y[:], y[:], OP.mult)
                    nc.vector.tensor_tensor(t[:], t[:], hv[:], OP.mult)
                    nc.vector.tensor_scalar(t[:], t[:], -1.0, 1.5,
                                            OP.mult, OP.add)
                    nc.vector.tensor_tensor(y[:], y[:], t[:], OP.mult)
                nc.vector.tensor_copy(ms_all[csl], y[:])

            def _xhat_scale(jt, p, qs):
                qb = jt * 2 + qs
                xh = tpool.tile([P, P], f16, name="xh")
                nc.vector.tensor_scalar_mul(xh[:], xat[p][:, qb, :],
                                            ms_all[:, qb, p:p + 1])
                pt = ptr.tile([P, P], f16, tag="pt", name="pt")
                nc.tensor.transpose(pt[:], xh[:], ident[:])
                nc.vector.tensor_copy(xhatT[p][:, qb, :], pt[:])

            def normalize_jt(jt):
                # batched rmsnorm rsqrt for this supertile's 10 row-groups
                _rsqrt_chain((slice(None), ds(2 * jt, 2), slice(None)),
                             2 * PAIRS)
                for qs in range(2):
                    for p in range(PAIRS):
                        _xhat_scale(jt, p, qs)

            def normalize_pair(jt, p):
                # per-pair variant (last supertile): xhatT finishes pair by
                # pair so the tail out-proj fills start without a stall
                _rsqrt_chain((slice(None), ds(2 * jt, 2), slice(p, p + 1)), 2)
                for qs in range(2):
                    _xhat_scale(jt, p, qs)

            def oproj_fill(qb, hc, pool, on_vec=False):
                po = pool.tile([P, 512], f32, tag="pj", name="po")
                for p in range(PAIRS):
                    nc.tensor.matmul(po[:], xhatT[p][:, qb, :],
                                     ow_sb[p][:, ds(hc * 512, 512)],
                                     start=(p == 0), stop=(p == PAIRS - 1))
                ot = opool.tile([P, 512], f16, name="ot")
                if on_vec:
                    nc.vector.tensor_copy(ot[:], po[:])
                else:
                    nc.scalar.copy(ot[:], po[:])
                nc.sync.dma_start(out_v[qb][:, ds(hc * 512, 512)], ot[:])

            # ---------- phase 3: attention ----------
            with (
                tc.tile_pool(name="psc", bufs=1, space="PSUM") as psc,
                tc.tile_pool(name="pav", bufs=1, space="PSUM") as pav,
            ):
                for jt in range(N_JT):
                    E = 4 * (jt + 1)          # key blocks for this supertile
                    prev_fills = ([(2 * (jt - 1) + f % 2, f // 2)
                                   for f in range(10)] if jt > 0 else [])
                    for p in range(PAIRS):
                        if jt == 0:
                            q_fill(1, p, pj)
                        if prev_fills:
                            oproj_fill(*prev_fills.pop(0), pj)
                            oproj_fill(*prev_fills.pop(0), pj, on_vec=True)
                        if sim_compat:
                            avt = [[pav.tile([P, 132], f32, tag=f"av{s}{q}",
                                             name=f"av{s}{q}") for q in range(2)]
                                   for s in range(2)]

                            def avap(s, qs, lo, n, avt=avt):
                                return avt[s][qs][:, ds(lo, n)]
                        else:
                            avt = [pav.tile([P, 264], f32, tag=f"av{s}",
                                            name=f"av{s}") for s in range(2)]

                            def avap(s, qs, lo, n, avt=avt):
                                return avt[s][:, ds(132 * qs + lo, n)]

                        def emit_av(ev, kbs):
                            for j, kb in enumerate(kbs):
                                for qs in range(2):
                                    st = kb == 0 and (qs == 0 or sim_compat)
                                    sp = kb == E - 1
                                    for s in range(2):
                                        nc.tensor.matmul(
                                            avap(s, qs, 0, 132),
                                            ev[:, s, j, ds(128 * qs, 128)],
                                            v_sb[:, kb, :], start=st, stop=sp)

                        pend = None
                        for grp in range(E // 4):     # 4 key blocks per group
                            kbs = tuple(range(4 * grp, 4 * grp + 4))
                            ee = etile.tile([P, 2, 4, QT], f16, name="ee")
                            rhs1 = qTall[0:64, p, ds(jt * QT, QT)]
                            rhs2 = qTall[64:128, p, ds(jt * QT, QT)]
                            for s, (ksl, rhs) in enumerate(
                                    ((slice(0, 64), rhs1),
                                     (slice(64, 128), rhs2))):
                                sc = psc.tile([P, 4, QT], f32, tag=f"s{s}",
                                              name=f"s{s}")
                                for j, kb in enumerate(kbs):
                                    nc.tensor.matmul(sc[:, j, :],
                                                     kT[ksl, ts(kb, P)],
                                                     rhs, start=True, stop=True)
                                nc.scalar.activation(ee[:, s], sc[:],
                                                     AF.Exp, scale=SCALE)
                            if grp == jt:             # last group = diag band
                                for s in range(2):
                                    nc.vector.tensor_tensor(ee[:, s], ee[:, s],
                                                            mask_sb[:], OP.mult)
                            if pend is not None:
                                emit_av(*pend)
                            pend = (ee, kbs)
                        emit_av(*pend)
                        # epilogue: combine + row sum-of-squares (rsqrt deferred)
                        rec = tpool.tile([P, 4], f32, name="rec")
                        if sim_compat:
                            for qs in range(2):
                                nc.vector.reciprocal(rec[:, qs:qs + 1],
                                                     avap(0, qs, 128, 1))
                                nc.vector.reciprocal(rec[:, 2 + qs:3 + qs],
                                                     avap(1, qs, 128, 1))
                        else:
                            for s in range(2):
                                nc.vector.reciprocal(
                                    rec[:, 2 * s:2 * s + 2],
                                    avt[s][:, ds(128, 2, 132)])
                        nc.vector.tensor_scalar_mul(rec[:, 2:4], rec[:, 2:4],
                                                    lam_sb[:])
                        for qs in range(2):
                            qb = jt * 2 + qs
                            xs = xat[p][:, qb, :]
                            xb = tpool.tile([P, P], f16, name="xb")
                            nc.vector.tensor_scalar_mul(
                                xs, avap(0, qs, 0, P), rec[:, qs:qs + 1])
                            nc.vector.tensor_scalar_mul(
                                xb[:], avap(1, qs, 0, P),
                                rec[:, 2 + qs:3 + qs])
                            nc.vector.tensor_tensor(xs, xs, xb[:], OP.subtract)
                            sq = tpool.tile([P, P], f32, name="sq")
                            nc.vector.tensor_tensor(sq[:], xs, xs, OP.mult)
                            nc.vector.reduce_sum(ms_all[:, qb, p:p + 1], sq[:],
                                                 axis=mybir.AxisListType.X)
                        if jt == N_JT - 1:
                            normalize_pair(jt, p)
                    if jt < N_JT - 1:
                        normalize_jt(jt)

            # ---------- final out-proj fills (last supertile) ----------
            with tc.tile_pool(name="pjT", bufs=3, space="PSUM") as pjT:
                for qb in (2 * (N_JT - 1), 2 * (N_JT - 1) + 1):
                    for hc in range(H // 512):
                        oproj_fill(qb, hc, pjT, on_vec=True)

    nc.compile()
    return nc


def _prep_inputs(hidden_states, Wqkv_w, Wqkv_b, out_w, out_b,
                 lambda_q1, lambda_k1, lambda_q2, lambda_k2, subln_w):
    hs = np.asarray(hidden_states, np.float32).reshape(S, H)
    Wqkv_w = np.asarray(Wqkv_w, np.float32)
    Wqkv_b = np.asarray(Wqkv_b, np.float32)
    out_w = np.asarray(out_w, np.float32)
    subln_w = np.asarray(subln_w, np.float32)

    lam_full = np.float32(
        np.exp(np.dot(np.asarray(lambda_q1, np.float64),
                      np.asarray(lambda_k1, np.float64)))
        - np.exp(np.dot(np.asarray(lambda_q2, np.float64),
                        np.asarray(lambda_k2, np.float64)))
        + LAMBDA_INIT)

    # Per seq-group hidden layout: pair-swap the sequence for g=1 so own
    # query columns always sit at even positions (one SPMD program). The
    # keys then appear pair-swapped too, which only the causal mask (also
    # per-core host data) needs to know about.
    hidTs, masks = [], []
    kk = np.arange(P)[:, None, None]
    bb = np.arange(4)[None, :, None]
    ii = np.arange(QT)[None, None, :]
    for g in range(N_SEQ):
        hidTs.append(np.ascontiguousarray(
            hs[np.arange(S) ^ g].T.astype(np.float16)))              # [H, S]
        masks.append(
            ((2 * ii + g) >= ((128 * bb + kk) ^ g)).astype(np.float16))

    in_maps = []
    for c in range(N_CORES):
        g, hg = c % N_SEQ, c // N_SEQ
        kp = hg // 2                                 # kv pair for this head group
        krows = slice(H + P * kp, H + P * (kp + 1))
        vrows = slice(H + NKV * D + P * kp, H + NKV * D + P * (kp + 1))
        qrows = slice(640 * hg, 640 * (hg + 1))
        wq = np.ascontiguousarray(Wqkv_w[qrows].T.astype(np.float16))   # [H, 640]
        wkv = np.ascontiguousarray(np.concatenate(
            [Wqkv_w[krows].T, Wqkv_w[vrows].T], axis=1).astype(np.float16))
        bkv = np.ascontiguousarray(
            np.stack([Wqkv_b[krows], Wqkv_b[vrows]], axis=1))            # [128,2]
        bq = np.ascontiguousarray(Wqkv_b[qrows].reshape(PAIRS, P).T)     # [128,5]
        sub = np.tile(subln_w, PAIRS) * (1.0 - LAMBDA_INIT)              # [640]
        ow = np.ascontiguousarray(
            (out_w[:, qrows].T * sub[:, None]).astype(np.float16))       # [640,H]
        in_maps.append({
            "hidT_kv": hidTs[g],
            "wkvT": wkv,
            "wqT": wq,
            "owT": ow,
            "bkv": bkv,
            "bq": bq,
            "maskd": masks[g],
            "lam": np.array([[lam_full]], np.float32),
        })
    return in_maps


def run(inputs, trace=False):
    global _PROGRAM
    if _PROGRAM is None:
        _PROGRAM = _build_program(
            sim_compat=os.environ.get("KSIMCOMPAT", "0") == "1")
    in_maps = _prep_inputs(**inputs)
    res = run_bass_kernel_spmd(_PROGRAM, in_maps,
                               core_ids=list(range(N_CORES)), trace=trace)
    out_b = np.asarray(inputs["out_b"], np.float32)
    full = np.empty((S, H), np.float32)
    for g in range(N_SEQ):
        acc = np.zeros((S_LOC, H), np.float32)
        for hg in range(N_HG):
            acc += np.asarray(res.results[hg * N_SEQ + g]["out"], np.float32)
        full[g::2] = acc + out_b
    return full.reshape(1, S, H), res


def kernel(**inputs):
    return run(inputs, trace=False)[0]

